# revision 5
# baseline (speedup 1.0000x reference)
"""Trainium2 Bass kernel for the Viterbi ACS step (nn_Link_21698174780141).

Reference computation:
    A  = in_prob @ (states_to_edges * states_to_edges_mask)   # [B, 128]
    Bm = llrs @ llrs_to_edges                                 # [B, 128]
    x  = (A + Bm).reshape(B, 64, 2)
    max_values = x.max(axis=2)                                # [B, 64] f32
    argmax     = x.argmax(axis=2)                             # [B, 64] int32

Strategy: pure data parallelism over the batch (65536 rows per core, 8 cores).
Per core, a single fused matmul computes x: stationary operand is the
transposed input tile [66, 128] (rows = 2 llr rows + 64 states), moving
operand is W = [llrs_to_edges ; s2e*mask] [66, 128] with columns permuted so
even edges land in psum columns 0..63 and odd edges in 64..127 of each
128-column block. The vector engine then does one pairwise max and one
is_gt over each [128, 512] psum supertile (4 batch tiles per matmul group).

Outputs: max_values in natural layout; argmax flags as uint8 in a
device-friendly layout, unscrambled + cast to int32 on the host.
"""

import json

import numpy as np

import concourse.bass as bass
import concourse.bass2jax as bass2jax
import concourse.mybir as mybir
import concourse.tile as tile
from concourse.bass_utils import run_bass_kernel_spmd

B = 524288
N_STATES = 64
N_EDGES = 128
RATE_INV = 2
N_CORES = 8
BS = B // N_CORES            # 65536 rows per core
TPS = 4                      # batch tiles (128 rows) per supertile
ROWS = 128 * TPS             # 512 rows per supertile
NST = BS // ROWS             # 128 supertiles per core
K = N_STATES + RATE_INV      # 66 contraction rows
LLRS_POS = "first"           # "first": W rows [llr0, llr1, s0..s63]

_WS_COUNT = [0]


def _split_sync_waits(bir_json, max_waits=1):
    """walrus in this container rejects instructions with >2 sem waits
    (setupSyncWait 'Too many sync wait commands'); hoist excess waits onto
    EventSemaphore instructions placed just before the offender on the same
    engine queue."""
    m = json.loads(bir_json)
    for f in m["functions"]:
        for bb in f["blocks"]:
            out = []
            for inst in bb["instructions"]:
                si = inst.get("sync_info")
                if si:
                    ow = si.get("on_wait") or []
                    while len(ow) > max_waits:
                        chunk, ow = ow[:max_waits], ow[max_waits:]
                        _WS_COUNT[0] += 1
                        out.append({
                            "engine": inst["engine"], "ins": [], "outs": [],
                            "name": f"waitsplit_{_WS_COUNT[0]}",
                            "opcode": "EventSemaphore",
                            "sync_info": {"on_update": [], "on_wait": chunk},
                        })
                    si["on_wait"] = ow
                out.append(inst)
            bb["instructions"] = out
    return json.dumps(m).encode()


_orig_cbk = bass2jax.compile_bir_kernel


def _patched_cbk(bir_json, tmpdir, neff_name="file.neff"):
    return _orig_cbk(_split_sync_waits(bir_json), tmpdir, neff_name=neff_name)


def _install_patch():
    if bass2jax.compile_bir_kernel is not _patched_cbk:
        bass2jax.compile_bir_kernel = _patched_cbk


def build_bass():
    nc = bass.Bass("TRN2", debug=False)
    lin = nc.dram_tensor("lin", [N_STATES, BS], mybir.dt.float32, kind="ExternalInput")
    llr = nc.dram_tensor("llr", [RATE_INV, BS], mybir.dt.float32, kind="ExternalInput")
    w = nc.dram_tensor("w", [K, N_EDGES], mybir.dt.float32, kind="ExternalInput")
    mv = nc.dram_tensor("mv", [BS, N_STATES], mybir.dt.float32, kind="ExternalOutput")
    fl = nc.dram_tensor("fl", [128, NST * TPS * 64], mybir.dt.uint8, kind="ExternalOutput")

    with tile.TileContext(nc) as tc:
        with (
            tc.tile_pool(name="const", bufs=1) as constp,
            tc.tile_pool(name="lhs", bufs=4) as lhsp,
            tc.tile_pool(name="psum", bufs=4, space=bass.MemorySpace.PSUM) as psump,
            tc.tile_pool(name="mvp", bufs=4) as mvp,
            tc.tile_pool(name="flp", bufs=4) as flp,
        ):
            w_sb = constp.tile([K, N_EDGES], mybir.dt.float32)
            nc.sync.dma_start(w_sb[:, :], w[:, :])

            for st in range(NST):
                c = st * ROWS
                lt = lhsp.tile([K, ROWS], mybir.dt.float32)
                if LLRS_POS == "first":
                    nc.sync.dma_start(lt[0:RATE_INV, :], llr[:, c:c + ROWS])
                    nc.sync.dma_start(lt[RATE_INV:K, :], lin[:, c:c + ROWS])
                else:
                    nc.sync.dma_start(lt[0:N_STATES, :], lin[:, c:c + ROWS])
                    nc.sync.dma_start(lt[N_STATES:K, :], llr[:, c:c + ROWS])

                pt = psump.tile([128, TPS * N_EDGES], mybir.dt.float32)
                for j in range(TPS):
                    nc.tensor.matmul(
                        pt[:, j * N_EDGES:(j + 1) * N_EDGES],
                        lt[:, j * 128:(j + 1) * 128],
                        w_sb[:, :],
                        start=True,
                        stop=True,
                    )

                v = pt[:, :].rearrange("p (j k d) -> p j k d", j=TPS, k=2)
                # DVE may read only one operand from PSUM per instruction:
                # stage the odd-edge half in SBUF via the scalar engine.
                ot = mvp.tile([128, TPS * 64], mybir.dt.float32, tag="odd")
                ot3 = ot[:, :].rearrange("p (j d) -> p j d", j=TPS)
                nc.scalar.copy(ot3, v[:, :, 1, :])
                mt = mvp.tile([128, TPS * 64], mybir.dt.float32)
                ft = flp.tile([128, TPS * 64], mybir.dt.uint8)
                mt3 = mt[:, :].rearrange("p (j d) -> p j d", j=TPS)
                ft3 = ft[:, :].rearrange("p (j d) -> p j d", j=TPS)
                nc.vector.tensor_tensor(
                    mt3, v[:, :, 0, :], ot3, op=mybir.AluOpType.max
                )
                nc.vector.tensor_tensor(
                    ft3, ot3, v[:, :, 0, :], op=mybir.AluOpType.is_gt
                )

                nc.scalar.dma_start(
                    mv[c:c + ROWS, :].rearrange("(j p) d -> p j d", p=128), mt3
                )
                nc.scalar.dma_start(fl[:, st * TPS * 64:(st + 1) * TPS * 64], ft[:, :])
    return nc


def _build_w(states_to_edges, states_to_edges_mask, llrs_to_edges):
    s2e = np.asarray(states_to_edges, np.float32) * np.asarray(
        states_to_edges_mask, np.float32
    )
    l2e = np.asarray(llrs_to_edges, np.float32)
    if LLRS_POS == "first":
        w = np.concatenate([l2e, s2e], axis=0)
    else:
        w = np.concatenate([s2e, l2e], axis=0)
    perm = np.concatenate([np.arange(0, N_EDGES, 2), np.arange(1, N_EDGES, 2)])
    return np.ascontiguousarray(w[:, perm])


def kernel(in_prob, llrs, states_to_edges, states_to_edges_mask, llrs_to_edges):
    _install_patch()
    in_prob = np.asarray(in_prob, np.float32)
    llrs = np.asarray(llrs, np.float32)
    w = _build_w(states_to_edges, states_to_edges_mask, llrs_to_edges)

    in_maps = []
    for s in range(N_CORES):
        sl = slice(s * BS, (s + 1) * BS)
        in_maps.append({
            "lin": np.ascontiguousarray(in_prob[sl].T),
            "llr": np.ascontiguousarray(llrs[sl].T),
            "w": w,
        })

    nc = build_bass()
    res = run_bass_kernel_spmd(nc, in_maps, core_ids=list(range(N_CORES)), trace=False)

    mv = np.concatenate([r["mv"] for r in res.results], axis=0)
    idx_shards = []
    for r in res.results:
        f = r["fl"].reshape(128, NST, TPS, 64)
        idx_shards.append(
            np.ascontiguousarray(f.transpose(1, 2, 0, 3)).reshape(BS, 64)
        )
    idx = np.concatenate(idx_shards, axis=0).astype(np.int32)
    return mv, idx
